# revision 60
# baseline (speedup 1.0000x reference)
"""Trainium2 Bass kernel for nn_LoRATACMLP4 (B=16,K=8,F=512,INCH=OUTCH=512,R=8).

Data-parallel over batch across 8 NeuronCores (2 batches per core).

Math (per batch b, slot k, token t):
    y    = mean_k(x @ W_ave.T) + b_ave          (mean commutes with linear)
    xp   = x @ W_pass.T + b_pass
    h    = gelu([xp, y])
    z    = h @ v / INCH ; lora = z @ u.T / R
    out  = gelu(h @ W_out.T + b_out + lora + b)

Device-side strategy:
  - x host-transposed to [inch, (k,f)] bf16; xsum (the k-sum) precomputed on
    host so the ave branch runs on F tokens instead of K*F.
  - ha (ave branch) is shared across k; its W_out contribution plus all
    output biases are computed ONCE per (batch, token-tile) as `base`
    [128t x 512o] and injected per (k-pair, token-tile) with a DVE add, so
    the per-k out matmul only contracts the pass half (2 chunks not 4).
  - z for a whole k-quad lands in one pre-zeroed PSUM bank (strip j =
    partitions 32j..32j+8), giving a single DVE copy to SBUF per quad; the
    rank-8 lora matmuls read those strips directly (u host-packed to match).
  - out-stage works on k-pairs: one [128,2,OUTCH] two-bank psum tile and one
    [128,4,2,OUTCH] staging tile per pair halve the DVE-add / ACT-gelu
    instruction count on the pacing-critical inject+gelu chain.
  - output stored bf16 (cast to f32 on host); 8 junk matmuls on resident
    weights at iteration start keep the PE HAM clock gate warm through the
    input-DMA wait; input DMA dispatch is split across the SP and Pool
    queues with the pass-critical first half of x in the smallest pieces.
"""

import sys

sys.path.insert(0, "/opt/trn_rl_repo")

import numpy as np
import ml_dtypes

BF16 = ml_dtypes.bfloat16
F8 = ml_dtypes.float8_e4m3
F8L = ml_dtypes.float8_e5m2

B, K, F, INCH, OUTCH, R = 16, 8, 512, 512, 512, 8
HD = INCH // 2
N_CORES = 8
BPC = B // N_CORES  # batches per core
KQ = K // 4  # k quads

_CACHE = {}

# bodies per hardware-loop iteration when timing (amortizes the For_i
# all-engine barrier); timing loop counts must be divisible by this.
TIME_UNROLL = 8


def _build_bass(loop_n=1, variant="", unroll=1):
    import contextlib
    import concourse.bass as bass
    import concourse.mybir as mybir
    from concourse import bacc, tile

    fp32 = mybir.dt.float32
    bf16 = mybir.dt.bfloat16
    AF = mybir.ActivationFunctionType

    nc = bacc.Bacc(None, target_bir_lowering=False)

    f8 = mybir.dt.float8e4
    f8l = mybir.dt.float8e5

    # x ships as a split-fp8 pair (x = x_hi + x_lo exactly in fp32): the big
    # GEMMs run fp8 DoubleRow (2 contraction chunks per instr, 0.5 cyc/row).
    # Layout [pair, p, cslot, kf]: partition p of slot cslot holds
    # inch = pair*256 + cslot*128 + p.
    xhi_d = nc.declare_dram_parameter("xhi", [BPC, 2, 128, 2, K * F], f8, isOutput=False)
    xlo_d = nc.declare_dram_parameter("xlo", [BPC, 2, 128, 2, K * F], f8, isOutput=False)
    xs_d = nc.declare_dram_parameter("xsum", [BPC, INCH, F], bf16, isOutput=False)
    # pass half of v only (rows 0..HD); the ave half ships as va2 strips.
    # fp8, UNSCALED (scale applied at the z psum->sbuf copy).
    v_d = nc.declare_dram_parameter("v", [BPC, HD, K * R], f8, isOutput=False)
    # ave-half v packed as a strip-placed stationary: one [128,128] block per
    # (chunk, quad) whose column 32*j+r holds v_ave[k=4kq+j, :, r], rest zero
    va2_d = nc.declare_dram_parameter("va2", [BPC, HD, KQ * 128], f8, isOutput=False)
    u2_d = nc.declare_dram_parameter("u2", [BPC, 128, KQ * OUTCH], bf16, isOutput=False)
    bv_d = nc.declare_dram_parameter("bvec", [BPC, 128, OUTCH], bf16, isOutput=False)
    # W_pass split-fp8 pair layout [pair, p, cslot, hd], pre-scaled by 16
    wph_d = nc.declare_dram_parameter("wpassh", [2, 128, 2, HD], f8, isOutput=False)
    wpl_d = nc.declare_dram_parameter("wpassl", [2, 128, 2, HD], f8l, isOutput=False)
    wa_d = nc.declare_dram_parameter("waveT", [INCH, HD], bf16, isOutput=False)
    # W_out pass half split-fp8 [p, cc, outch] (contraction pair = cc), x16;
    # ave half stays bf16 (base path), x16
    woh_d = nc.declare_dram_parameter("wouth", [128, 2, OUTCH], f8, isOutput=False)
    wol_d = nc.declare_dram_parameter("woutl", [128, 2, OUTCH], f8l, isOutput=False)
    wo_d = nc.declare_dram_parameter("woutT", [HD, OUTCH], bf16, isOutput=False)
    bp_d = nc.declare_dram_parameter("bpass", [HD, 1], fp32, isOutput=False)
    ba_d = nc.declare_dram_parameter("bave", [HD, 1], fp32, isOutput=False)
    out_d = nc.declare_dram_parameter("out", [BPC, K, F, OUTCH], bf16, isOutput=True)

    with tile.TileContext(nc) as tc:
        with (
            tc.tile_pool(name="consts", bufs=1) as cpool,
            tc.tile_pool(name="xt", bufs=16) as xt_pool,
            tc.tile_pool(name="xs", bufs=8) as xs_pool,
            tc.tile_pool(name="vt", bufs=8) as vt_pool,
            tc.tile_pool(name="u2", bufs=2) as u2_pool,
            tc.tile_pool(name="bv", bufs=2) as bv_pool,
            tc.tile_pool(name="ha", bufs=2) as ha_pool,
            tc.tile_pool(name="ha8", bufs=2) as ha8_pool,
            tc.tile_pool(name="bs", bufs=8) as base_pool,
            tc.tile_pool(name="hp", bufs=8) as hp_pool,
            tc.tile_pool(name="hp8", bufs=8) as hp8_pool,
            tc.tile_pool(name="zs", bufs=2) as zsb_pool,
            tc.tile_pool(name="ta", bufs=4) as ta_pool,
            tc.tile_pool(name="osb", bufs=4) as osb_pool,
            tc.tile_pool(name="ps_mm", bufs=3, space="PSUM") as ps_mm,
            tc.tile_pool(name="ps_z", bufs=1, space="PSUM") as ps_z,
            tc.tile_pool(name="ps_o", bufs=2, space="PSUM") as ps_o,
        ):
            # persistent weights / biases
            wout_sb, wa_sb, wph_sb, wpl_sb = [], [], [], []
            for c in range(2):
                w = cpool.tile([128, OUTCH], bf16, name=f"wo{c}", tag=f"wo{c}")
                nc.sync.dma_start(out=w[:], in_=wo_d[c * 128 : (c + 1) * 128, :])
                wout_sb.append(w)
                w = cpool.tile([128, 2, HD], f8, name=f"wph{c}", tag=f"wph{c}")
                nc.sync.dma_start(out=w[:], in_=wph_d[c])
                wph_sb.append(w)
                w = cpool.tile([128, 2, HD], f8l, name=f"wpl{c}", tag=f"wpl{c}")
                nc.sync.dma_start(out=w[:], in_=wpl_d[c])
                wpl_sb.append(w)
            for c in range(4):
                w = cpool.tile([128, HD], bf16, name=f"wa{c}", tag=f"wa{c}")
                nc.sync.dma_start(out=w[:], in_=wa_d[c * 128 : (c + 1) * 128, :])
                wa_sb.append(w)
            woh_sb = cpool.tile([128, 2, OUTCH], f8, name="woh", tag="woh")
            nc.sync.dma_start(out=woh_sb[:], in_=woh_d[:])
            wol_sb = cpool.tile([128, 2, OUTCH], f8l, name="wol", tag="wol")
            nc.sync.dma_start(out=wol_sb[:], in_=wol_d[:])
            bp_sb, ba_sb = [], []
            for m in range(2):
                t = cpool.tile([128, 1], fp32, name=f"bp{m}", tag=f"bp{m}")
                nc.sync.dma_start(out=t[:], in_=bp_d[m * 128 : (m + 1) * 128, :])
                bp_sb.append(t)
                t = cpool.tile([128, 1], fp32, name=f"ba{m}", tag=f"ba{m}")
                nc.sync.dma_start(out=t[:], in_=ba_d[m * 128 : (m + 1) * 128, :])
                ba_sb.append(t)

            def emit_prewarm():
                # junk matmuls on resident weights during the start-of-
                # iteration DMA wait: PE is idle anyway and ~3.5us of busy
                # work flips the HAM clock gate to 8/8 before real work lands.
                warm = ps_mm.tile([128, OUTCH], fp32, tag="mm", name="warm")
                for i in range(10):
                    nc.tensor.matmul(
                        warm[:],
                        wa_sb[i % 4][:, 0:128],
                        wout_sb[i % 2][:],
                        start=(i == 0),
                        stop=(i == 9),
                    )

            def emit_batch(b):
                # batch 0 splits input dispatch across the idle SP queue and
                # Pool so the ~0.6us/DMA descriptor-gen cost parallelizes at
                # iteration start; x is split into k-halves so pass k=0..3
                # only waits on the first half.
                hot = nc.sync if b == 0 else nc.gpsimd
                xss = []
                for c in range(4):
                    xs = xs_pool.tile([128, F], bf16, tag="xs", name=f"xs{b}_{c}")
                    hot.dma_start(out=xs[:], in_=xs_d[b, c * 128 : (c + 1) * 128, :])
                    xss.append(xs)
                xth = {}
                for h in range(2):
                    for pair in range(2):
                        for lohi, xd in (("h", xhi_d), ("l", xlo_d)):
                            xt = xt_pool.tile(
                                [128, 2, 4 * F],
                                f8,
                                tag="xt",
                                name=f"xt{b}{h}{pair}{lohi}",
                            )
                            if h == 0:
                                # split the pass-critical first half across
                                # two dispatch queues and twice the DMA queues
                                eng = hot if pair == 0 else nc.gpsimd
                                for q in range(2):
                                    eng.dma_start(
                                        out=xt[:, :, q * 2 * F : (q + 1) * 2 * F],
                                        in_=xd[b, pair][
                                            :, :, q * 2 * F : (q + 1) * 2 * F
                                        ],
                                    )
                            else:
                                nc.gpsimd.dma_start(
                                    out=xt[:],
                                    in_=xd[b, pair][:, :, 4 * F : 8 * F],
                                )
                            xth[(h, pair, lohi)] = xt
                bvt = bv_pool.tile([128, OUTCH], bf16, tag="bv", name=f"bv{b}")
                nc.gpsimd.dma_start(out=bvt[:], in_=bv_d[b])
                vts = []
                for c in range(2):
                    vt = vt_pool.tile([128, K * R], f8, tag="vt", name=f"vt{b}_{c}")
                    nc.gpsimd.dma_start(
                        out=vt[:], in_=v_d[b, c * 128 : (c + 1) * 128, :]
                    )
                    vts.append(vt)
                va2t = []
                for c in range(2):
                    va = vt_pool.tile(
                        [128, KQ * 128], f8, tag="va2", name=f"va2{b}_{c}"
                    )
                    nc.gpsimd.dma_start(
                        out=va[:], in_=va2_d[b, c * 128 : (c + 1) * 128, :]
                    )
                    va2t.append(va)
                u2t = u2_pool.tile([128, KQ * OUTCH], bf16, tag="u2", name=f"u2_{b}")
                nc.gpsimd.dma_start(out=u2t[:], in_=u2_d[b])

                # ave branch on k-summed tokens: ha = gelu(Wa/K @ xsum + ba)
                hat = ha_pool.tile([128, 2, F], bf16, tag="ha", name=f"ha{b}")
                for m in range(2):
                    ps = ps_mm.tile([128, F], fp32, tag="mm", name=f"psy{b}_{m}")
                    for c in range(4):
                        nc.tensor.matmul(
                            ps[:],
                            wa_sb[c][:, m * 128 : (m + 1) * 128],
                            xss[c][:],
                            start=(c == 0),
                            stop=(c == 3),
                        )
                    nc.scalar.activation(hat[:, m, :], ps[:], AF.Gelu, bias=ba_sb[m][:])
                ha8 = ha8_pool.tile([128, 2, F], f8, tag="ha8", name=f"ha8{b}")
                nc.vector.tensor_copy(ha8[:], hat[:])

                # base[mt] = ha @ Wout[256:512] + (b_out + b[batch])  per token tile
                bases = []
                for mt in range(4):
                    ps = ps_mm.tile([128, OUTCH], fp32, tag="mm", name=f"psb{b}_{mt}")
                    nc.tensor.matmul(
                        ps[:],
                        hat[:, 0, mt * 128 : (mt + 1) * 128],
                        wout_sb[0][:],
                        start=True,
                        stop=False,
                    )
                    nc.tensor.matmul(
                        ps[:],
                        hat[:, 1, mt * 128 : (mt + 1) * 128],
                        wout_sb[1][:],
                        start=False,
                        stop=True,
                    )
                    bs = base_pool.tile(
                        [128, OUTCH], bf16, tag="bs", name=f"bs{b}_{mt}"
                    )
                    nc.vector.tensor_add(bs[:], ps[:], bvt[:])
                    bases.append(bs)

                for kq in range(KQ):
                    # pass branch for the quad: hp = gelu(Wp @ x + bp).
                    # Both hd-halves of a k share one [128,2,F] tile so the
                    # fp8 shadow copy (z-matmul moving operand) is one DVE op.
                    hps = {}
                    hp8s = {}
                    hpls = {}
                    for j in range(4):
                        k = 4 * kq + j
                        hpj = hp_pool.tile(
                            [128, 2, F], bf16, tag="hp", name=f"hp{b}{k}"
                        )
                        hp8 = hp8_pool.tile(
                            [128, 2, F], f8, tag="hp8", name=f"hp8{b}{k}"
                        )
                        for m in range(2):
                            ps = ps_mm.tile(
                                [128, F], fp32, tag="mm", name=f"psp{b}{k}{m}"
                            )
                            nmm = 0
                            for xt8, wsb in (
                                ("h", wph_sb),
                                ("l", wph_sb),
                                ("h", wpl_sb),
                            ):
                                for pair in range(2):
                                    nc.tensor.matmul(
                                        ps[:],
                                        wsb[pair][:, :, m * 128 : (m + 1) * 128],
                                        xth[(k // 4, pair, xt8)][
                                            :, :, (k % 4) * F : (k % 4 + 1) * F
                                        ],
                                        start=(nmm == 0),
                                        stop=(nmm == 5),
                                        perf_mode=mybir.MatmulPerfMode.DoubleRow,
                                    )
                                    nmm += 1
                            nc.scalar.activation(
                                hpj[:, m, :],
                                ps[:],
                                AF.Gelu,
                                bias=bp_sb[m][:],
                                scale=1.0 / 16.0,
                            )
                        nc.vector.tensor_copy(hp8[:], hpj[:])
                        hpl = hp8_pool.tile(
                            [128, 2, F], f8, tag="hpl", name=f"hpl{b}{k}"
                        )
                        nc.vector.tensor_sub(hpl[:], hpj[:], hp8[:])
                        hps[j] = hpj
                        hp8s[j] = hp8
                        hpls[j] = hpl

                    # z for the whole quad, col-tiled: strip j holds z_{4kq+j}.
                    # The ave half is batched: one full-width matmul per chunk
                    # with the strip-placed va2 stationary covers all 4 k at
                    # once; start=True on the first also zeroes the bank (its
                    # output AP spans all 128 partitions), so no explicit
                    # zeroing op is needed. The pass half then accumulates
                    # into the strips.
                    zq = ps_z.tile([128, F], fp32, tag="zq", name=f"zq{b}{kq}")
                    for cc in range(2):
                        nc.tensor.matmul(
                            zq[:],
                            va2t[cc][:, kq * 128 : (kq + 1) * 128],
                            ha8[:, cc, :],
                            start=(cc == 0),
                            stop=False,
                            skip_group_check=True,
                        )
                    cj = [(c, j) for c in range(2) for j in range(4)]
                    for nmm, (c, j) in enumerate(cj):
                        k = 4 * kq + j
                        nc.tensor.matmul(
                            zq[32 * j : 32 * j + 8, :],
                            vts[c][:, k * R : (k + 1) * R],
                            hp8s[j][:, c, :],
                            start=False,
                            stop=(nmm == 7),
                            tile_position=(0, 32 * j),
                            skip_group_check=True,
                        )
                    zsb = zsb_pool.tile([128, F], bf16, tag="zs", name=f"zs{b}{kq}")
                    nc.vector.tensor_scalar_mul(zsb[:], zq[:], 1.0 / 4096.0)

                    # k-pairs share a 2-bank psum tile and a [128,4,2,OUTCH]
                    # staging tile, halving DVE/ACT instruction count in the
                    # pace-critical inject+gelu chain.
                    osb2 = []
                    for jj in range(2):
                        osb2.append(
                            osb_pool.tile(
                                [128, 4, 2, OUTCH],
                                bf16,
                                tag="osb",
                                name=f"o{b}{kq}{jj}",
                            )
                        )
                    for mt in range(4):
                        po2s = []
                        for jj in range(2):
                            po2 = ps_o.tile(
                                [128, 2, OUTCH], fp32, tag="po", name=f"po{b}{kq}{mt}{jj}"
                            )
                            for sub in range(2):
                                j = 2 * jj + sub
                                for nmm, (hx, wx) in enumerate(
                                    (
                                        (hp8s[j], woh_sb),
                                        (hpls[j], woh_sb),
                                        (hp8s[j], wol_sb),
                                    )
                                ):
                                    nc.tensor.matmul(
                                        po2[:, sub, :],
                                        hx[:, :, mt * 128 : (mt + 1) * 128],
                                        wx[:],
                                        start=(nmm == 0),
                                        stop=False,
                                        perf_mode=mybir.MatmulPerfMode.DoubleRow,
                                    )
                            po2s.append(po2)
                        for j in range(4):
                            nc.tensor.matmul(
                                po2s[j // 2][:, j % 2, :],
                                zsb[32 * j : 32 * j + 8, mt * 128 : (mt + 1) * 128],
                                u2t[
                                    32 * j : 32 * j + 8,
                                    kq * OUTCH : (kq + 1) * OUTCH,
                                ],
                                start=False,
                                stop=True,
                                tile_position=(32 * j, 0),
                            )
                        bcast = (
                            bases[mt][:]
                            .unsqueeze(1)
                            .broadcast_to((128, 2, OUTCH))
                        )
                        for jj in range(2):
                            ta2 = ta_pool.tile(
                                [128, 2, OUTCH],
                                fp32,
                                tag="ta",
                                name=f"ta{b}{kq}{mt}{jj}",
                            )
                            nc.vector.tensor_add(ta2[:], po2s[jj][:], bcast)
                            nc.scalar.activation(
                                osb2[jj][:, mt, :, :],
                                ta2[:],
                                AF.Gelu,
                                scale=1.0 / 16.0,
                            )
                    for j in range(4):
                        k = 4 * kq + j
                        oview = out_d[b, k].rearrange("(m p) o -> p m o", p=128)
                        for hh in range(2):
                            nc.sync.dma_start(
                                out=oview[:, 2 * hh : 2 * hh + 2, :],
                                in_=osb2[j // 2][:, 2 * hh : 2 * hh + 2, j % 2, :],
                            )

            # `unroll` bodies per hardware-loop iteration: amortizes the
            # For_i all-engine barrier (head/tail pipeline drain) over
            # `unroll` iterations.
            assert loop_n % unroll == 0, (loop_n, unroll)
            n_hw = loop_n // unroll
            loop_cm = tc.For_i(0, n_hw, 1) if n_hw > 1 else contextlib.nullcontext()
            with loop_cm:
                for _u in range(unroll):
                    if _u == 0:
                        # only the first body per hw-iteration needs the
                        # HAM-warm: later bodies chain PE work back-to-back
                        emit_prewarm()
                    for b in range(BPC):
                        emit_batch(b)
    nc.compile()
    return nc


def _prep_inputs(x, u, v, b, W_pass, b_pass, W_ave, b_ave, W_out, b_out):
    x = np.asarray(x, dtype=np.float32)
    u = np.asarray(u, dtype=np.float32)
    v = np.asarray(v, dtype=np.float32)
    b = np.asarray(b, dtype=np.float32)

    # split-fp8 x: x = x_hi + x_lo (exact in fp32 up to x_lo's own e4m3
    # rounding). Layout [pair, p, cslot, kf]: inch = pair*256 + cslot*128 + p.
    xT = x.reshape(B, K * F, INCH).transpose(0, 2, 1)  # [B, INCH, KF]
    xr = xT.reshape(B, 2, 2, 128, K * F).transpose(0, 1, 3, 2, 4)
    xhi = xr.astype(F8)
    xlo = (xr - xhi.astype(np.float32)).astype(F8)
    xsum = np.ascontiguousarray(x.sum(axis=1).transpose(0, 2, 1)).astype(BF16)
    # v unscaled fp8; the 1/(INCH*R) lands on the z psum->sbuf copy
    vs = v.transpose(0, 2, 1, 3)  # [B, INCH, K, R]
    vb = np.ascontiguousarray(vs[:, :HD].reshape(B, HD, K * R)).astype(F8)
    # va2[b, i, kq*128 + 32*j + r] = v[b, HD + i, 4*kq + j, r]
    va2 = np.zeros((B, HD, KQ * 128), dtype=np.float32)
    for kq in range(KQ):
        for j in range(4):
            va2[:, :, kq * 128 + 32 * j : kq * 128 + 32 * j + 8] = vs[
                :, HD:, 4 * kq + j, :
            ]
    va2 = va2.astype(F8)
    # u2[b, 32*j + r, kq*OUTCH + o] = 16 * u[b, 4*kq + j, o, r]  (the out
    # psum runs at 16x; the final gelu rescales by 1/16)
    u2 = np.zeros((B, 128, KQ * OUTCH), dtype=BF16)
    ut = (u * 16.0).transpose(0, 1, 3, 2).astype(BF16)  # [B, K, R, OUTCH]
    for kq in range(KQ):
        for j in range(4):
            u2[:, 32 * j : 32 * j + 8, kq * OUTCH : (kq + 1) * OUTCH] = ut[
                :, 4 * kq + j
            ]
    bvec = (
        16.0 * (np.asarray(b_out, np.float32)[None, :] + b[:, 0, 0, :])
    ).astype(BF16)
    bvec128 = np.ascontiguousarray(
        np.broadcast_to(bvec[:, None, :], (B, 128, OUTCH))
    )
    # W_pass split-fp8 (x16): [pair, p, cslot, hd]
    wpT = 16.0 * np.asarray(W_pass, dtype=np.float32).T  # [INCH, HD]
    wpr = wpT.reshape(2, 2, 128, HD).transpose(0, 2, 1, 3)
    wpassh = wpr.astype(F8)
    wpassl = (wpr - wpassh.astype(np.float32)).astype(F8L)
    waveT = np.ascontiguousarray(np.asarray(W_ave, dtype=np.float32).T / K).astype(BF16)
    # W_out (x16): pass half split-fp8 [p, cc, outch]; ave half bf16
    woT = 16.0 * np.asarray(W_out, dtype=np.float32).T  # [INCH, OUTCH]
    wor = woT[:HD].reshape(2, 128, OUTCH).transpose(1, 0, 2)
    wouth = wor.astype(F8)
    woutl = (wor - wouth.astype(np.float32)).astype(F8L)
    woutT = np.ascontiguousarray(woT[HD:]).astype(BF16)
    bp = np.asarray(b_pass, dtype=np.float32).reshape(HD, 1)
    ba = np.asarray(b_ave, dtype=np.float32).reshape(HD, 1)

    in_maps = []
    for i in range(N_CORES):
        sl = slice(i * BPC, (i + 1) * BPC)
        in_maps.append(
            dict(
                xhi=np.ascontiguousarray(xhi[sl]),
                xlo=np.ascontiguousarray(xlo[sl]),
                xsum=np.ascontiguousarray(xsum[sl]),
                v=np.ascontiguousarray(vb[sl]),
                va2=np.ascontiguousarray(va2[sl]),
                u2=np.ascontiguousarray(u2[sl]),
                bvec=np.ascontiguousarray(bvec128[sl]),
                wpassh=wpassh,
                wpassl=wpassl,
                waveT=waveT,
                wouth=wouth,
                woutl=woutl,
                woutT=woutT,
                bpass=bp,
                bave=ba,
            )
        )
    return in_maps


def run(inputs, trace=False, loop_n=1, **spmd_kwargs):
    from concourse.bass_utils import run_bass_kernel_spmd

    key = ("nc", loop_n)
    if key not in _CACHE:
        _CACHE[key] = _build_bass(loop_n)
    nc = _CACHE[key]
    in_maps = _prep_inputs(**inputs)
    res = run_bass_kernel_spmd(
        nc, in_maps, list(range(N_CORES)), trace=trace, **spmd_kwargs
    )
    out = np.concatenate(
        [np.asarray(res.results[i]["out"]).astype(np.float32) for i in range(N_CORES)],
        axis=0,
    ).reshape(B, K, F, OUTCH)
    return out, res


def kernel(**inputs):
    out, _ = run(inputs, trace=False)
    return out



# revision 61
# speedup vs baseline: 1.7470x; 1.7470x over previous
"""Trainium2 Bass kernel for nn_LoRATACMLP4 (B=16,K=8,F=512,INCH=OUTCH=512,R=8).

Data-parallel over batch across 8 NeuronCores (2 batches per core).

Math (per batch b, slot k, token t):
    y    = mean_k(x @ W_ave.T) + b_ave          (mean commutes with linear)
    xp   = x @ W_pass.T + b_pass
    h    = gelu([xp, y])
    z    = h @ v / INCH ; lora = z @ u.T / R
    out  = gelu(h @ W_out.T + b_out + lora + b)

Device-side strategy:
  - x host-transposed to [inch, (k,f)] bf16; xsum (the k-sum) precomputed on
    host so the ave branch runs on F tokens instead of K*F.
  - ha (ave branch) is shared across k; its W_out contribution plus all
    output biases are computed ONCE per (batch, token-tile) as `base`
    [128t x 512o] and injected per (k-pair, token-tile) with a DVE add, so
    the per-k out matmul only contracts the pass half (2 chunks not 4).
  - z for a whole k-quad lands in one pre-zeroed PSUM bank (strip j =
    partitions 32j..32j+8), giving a single DVE copy to SBUF per quad; the
    rank-8 lora matmuls read those strips directly (u host-packed to match).
  - out-stage works on k-pairs: one [128,2,OUTCH] two-bank psum tile and one
    [128,4,2,OUTCH] staging tile per pair halve the DVE-add / ACT-gelu
    instruction count on the pacing-critical inject+gelu chain.
  - output stored bf16 (cast to f32 on host); 8 junk matmuls on resident
    weights at iteration start keep the PE HAM clock gate warm through the
    input-DMA wait; input DMA dispatch is split across the SP and Pool
    queues with the pass-critical first half of x in the smallest pieces.
"""

import sys

sys.path.insert(0, "/opt/trn_rl_repo")

import numpy as np
import ml_dtypes

BF16 = ml_dtypes.bfloat16
F8 = ml_dtypes.float8_e4m3
F8L = ml_dtypes.float8_e5m2

B, K, F, INCH, OUTCH, R = 16, 8, 512, 512, 512, 8
HD = INCH // 2
N_CORES = 8
BPC = B // N_CORES  # batches per core
KQ = K // 4  # k quads

_CACHE = {}

# bodies per hardware-loop iteration when timing (amortizes the For_i
# all-engine barrier); timing loop counts must be divisible by this.
TIME_UNROLL = 8


def _build_bass(loop_n=1, variant="", unroll=1):
    import contextlib
    import concourse.bass as bass
    import concourse.mybir as mybir
    from concourse import bacc, tile

    fp32 = mybir.dt.float32
    bf16 = mybir.dt.bfloat16
    AF = mybir.ActivationFunctionType

    nc = bacc.Bacc(None, target_bir_lowering=False)

    f8 = mybir.dt.float8e4
    f8l = mybir.dt.float8e5

    # x ships as a split-fp8 pair (x = x_hi + x_lo exactly in fp32): the big
    # GEMMs run fp8 DoubleRow (2 contraction chunks per instr, 0.5 cyc/row).
    # Layout [pair, p, cslot, kf]: partition p of slot cslot holds
    # inch = pair*256 + cslot*128 + p.
    x_d = nc.declare_dram_parameter("x", [BPC, INCH, K * F], bf16, isOutput=False)
    xhi_d = nc.declare_dram_parameter("xhi", [BPC, 2, 128, 2, K * F], f8, isOutput=False)
    xlo_d = nc.declare_dram_parameter("xlo", [BPC, 2, 128, 2, K * F], f8, isOutput=False)
    xs_d = nc.declare_dram_parameter("xsum", [BPC, INCH, F], bf16, isOutput=False)
    # pass half of v only (rows 0..HD); the ave half ships as va2 strips.
    # fp8, UNSCALED (scale applied at the z psum->sbuf copy).
    v_d = nc.declare_dram_parameter("v", [BPC, HD, K * R], bf16, isOutput=False)
    # ave-half v packed as a strip-placed stationary: one [128,128] block per
    # (chunk, quad) whose column 32*j+r holds v_ave[k=4kq+j, :, r], rest zero
    va2_d = nc.declare_dram_parameter("va2", [BPC, HD, KQ * 128], bf16, isOutput=False)
    u2_d = nc.declare_dram_parameter("u2", [BPC, 128, KQ * OUTCH], bf16, isOutput=False)
    bv_d = nc.declare_dram_parameter("bvec", [BPC, 128, OUTCH], bf16, isOutput=False)
    # W_pass split-fp8 pair layout [pair, p, cslot, hd], pre-scaled by 16
    wp_d = nc.declare_dram_parameter("wpassT", [INCH, HD], bf16, isOutput=False)
    wph_d = nc.declare_dram_parameter("wpassh", [2, 128, 2, HD], f8, isOutput=False)
    wpl_d = nc.declare_dram_parameter("wpassl", [2, 128, 2, HD], f8l, isOutput=False)
    wa_d = nc.declare_dram_parameter("waveT", [INCH, HD], bf16, isOutput=False)
    # W_out pass half split-fp8 [p, cc, outch] (contraction pair = cc), x16;
    # ave half stays bf16 (base path), x16
    woh_d = nc.declare_dram_parameter("wouth", [128, 2, OUTCH], f8, isOutput=False)
    wol_d = nc.declare_dram_parameter("woutl", [128, 2, OUTCH], f8l, isOutput=False)
    wo_d = nc.declare_dram_parameter("woutT", [INCH, OUTCH], bf16, isOutput=False)
    bp_d = nc.declare_dram_parameter("bpass", [HD, 1], fp32, isOutput=False)
    ba_d = nc.declare_dram_parameter("bave", [HD, 1], fp32, isOutput=False)
    out_d = nc.declare_dram_parameter("out", [BPC, K, F, OUTCH], bf16, isOutput=True)

    with tile.TileContext(nc) as tc:
        with (
            tc.tile_pool(name="consts", bufs=1) as cpool,
            tc.tile_pool(name="xt", bufs=16) as xt_pool,
            tc.tile_pool(name="xs", bufs=8) as xs_pool,
            tc.tile_pool(name="vt", bufs=8) as vt_pool,
            tc.tile_pool(name="u2", bufs=2) as u2_pool,
            tc.tile_pool(name="bv", bufs=2) as bv_pool,
            tc.tile_pool(name="ha", bufs=2) as ha_pool,
            tc.tile_pool(name="ha8", bufs=2) as ha8_pool,
            tc.tile_pool(name="bs", bufs=8) as base_pool,
            tc.tile_pool(name="hp", bufs=8) as hp_pool,
            tc.tile_pool(name="hp8", bufs=8) as hp8_pool,
            tc.tile_pool(name="zs", bufs=2) as zsb_pool,
            tc.tile_pool(name="ta", bufs=4) as ta_pool,
            tc.tile_pool(name="osb", bufs=4) as osb_pool,
            tc.tile_pool(name="ps_mm", bufs=3, space="PSUM") as ps_mm,
            tc.tile_pool(name="ps_z", bufs=1, space="PSUM") as ps_z,
            tc.tile_pool(name="ps_o", bufs=2, space="PSUM") as ps_o,
        ):
            # persistent weights / biases
            wout_sb, wa_sb, wph_sb, wpl_sb = [], [], [], []
            for c in range(4):
                w = cpool.tile([128, OUTCH], bf16, name=f"wo{c}", tag=f"wo{c}")
                nc.sync.dma_start(out=w[:], in_=wo_d[c * 128 : (c + 1) * 128, :])
                wout_sb.append(w)
            for c in range(2):
                w = cpool.tile([128, 2, HD], f8, name=f"wph{c}", tag=f"wph{c}")
                nc.sync.dma_start(out=w[:], in_=wph_d[c])
                wph_sb.append(w)
                w = cpool.tile([128, 2, HD], f8l, name=f"wpl{c}", tag=f"wpl{c}")
                nc.sync.dma_start(out=w[:], in_=wpl_d[c])
                wpl_sb.append(w)
            for c in range(4):
                w = cpool.tile([128, HD], bf16, name=f"wa{c}", tag=f"wa{c}")
                nc.sync.dma_start(out=w[:], in_=wa_d[c * 128 : (c + 1) * 128, :])
                wa_sb.append(w)
            wp_sb = []
            for c in range(4):
                w = cpool.tile([128, HD], bf16, name=f"wp{c}", tag=f"wp{c}")
                nc.sync.dma_start(out=w[:], in_=wp_d[c * 128 : (c + 1) * 128, :])
                wp_sb.append(w)
            woh_sb = cpool.tile([128, 2, OUTCH], f8, name="woh", tag="woh")
            nc.sync.dma_start(out=woh_sb[:], in_=woh_d[:])
            wol_sb = cpool.tile([128, 2, OUTCH], f8l, name="wol", tag="wol")
            nc.sync.dma_start(out=wol_sb[:], in_=wol_d[:])
            bp_sb, ba_sb = [], []
            for m in range(2):
                t = cpool.tile([128, 1], fp32, name=f"bp{m}", tag=f"bp{m}")
                nc.sync.dma_start(out=t[:], in_=bp_d[m * 128 : (m + 1) * 128, :])
                bp_sb.append(t)
                t = cpool.tile([128, 1], fp32, name=f"ba{m}", tag=f"ba{m}")
                nc.sync.dma_start(out=t[:], in_=ba_d[m * 128 : (m + 1) * 128, :])
                ba_sb.append(t)

            def emit_prewarm():
                # junk matmuls on resident weights during the start-of-
                # iteration DMA wait: PE is idle anyway and ~3.5us of busy
                # work flips the HAM clock gate to 8/8 before real work lands.
                warm = ps_mm.tile([128, OUTCH], fp32, tag="mm", name="warm")
                for i in range(10):
                    nc.tensor.matmul(
                        warm[:],
                        wa_sb[i % 4][:, 0:128],
                        wout_sb[i % 2][:],
                        start=(i == 0),
                        stop=(i == 9),
                    )

            def emit_batch(b):
                # batch 0 splits input dispatch across the idle SP queue and
                # Pool so the ~0.6us/DMA descriptor-gen cost parallelizes at
                # iteration start; x is split into k-halves so pass k=0..3
                # only waits on the first half.
                hot = nc.sync if b == 0 else nc.gpsimd
                xss = []
                for c in range(4):
                    xs = xs_pool.tile([128, F], bf16, tag="xs", name=f"xs{b}_{c}")
                    hot.dma_start(out=xs[:], in_=xs_d[b, c * 128 : (c + 1) * 128, :])
                    xss.append(xs)
                xth = {}
                for h in range(2):
                    for c in range(4):
                        xt = xt_pool.tile(
                            [128, 4 * F], bf16, tag="xt", name=f"xt{b}_{h}_{c}"
                        )
                        if h == 0:
                            eng = hot if c < 2 else nc.gpsimd
                            for q in range(2):
                                eng.dma_start(
                                    out=xt[:, q * 2 * F : (q + 1) * 2 * F],
                                    in_=x_d[
                                        b,
                                        c * 128 : (c + 1) * 128,
                                        q * 2 * F : (q + 1) * 2 * F,
                                    ],
                                )
                        else:
                            nc.gpsimd.dma_start(
                                out=xt[:],
                                in_=x_d[b, c * 128 : (c + 1) * 128, 4 * F : 8 * F],
                            )
                        xth[(h, c)] = xt
                bvt = bv_pool.tile([128, OUTCH], bf16, tag="bv", name=f"bv{b}")
                nc.gpsimd.dma_start(out=bvt[:], in_=bv_d[b])
                vts = []
                for c in range(2):
                    vt = vt_pool.tile([128, K * R], bf16, tag="vt", name=f"vt{b}_{c}")
                    nc.gpsimd.dma_start(
                        out=vt[:], in_=v_d[b, c * 128 : (c + 1) * 128, :]
                    )
                    vts.append(vt)
                va2t = []
                for c in range(2):
                    va = vt_pool.tile(
                        [128, KQ * 128], bf16, tag="va2", name=f"va2{b}_{c}"
                    )
                    nc.gpsimd.dma_start(
                        out=va[:], in_=va2_d[b, c * 128 : (c + 1) * 128, :]
                    )
                    va2t.append(va)
                u2t = u2_pool.tile([128, KQ * OUTCH], bf16, tag="u2", name=f"u2_{b}")
                nc.gpsimd.dma_start(out=u2t[:], in_=u2_d[b])

                # ave branch on k-summed tokens: ha = gelu(Wa/K @ xsum + ba)
                hat = ha_pool.tile([128, 2, F], bf16, tag="ha", name=f"ha{b}")
                for m in range(2):
                    ps = ps_mm.tile([128, F], fp32, tag="mm", name=f"psy{b}_{m}")
                    for c in range(4):
                        nc.tensor.matmul(
                            ps[:],
                            wa_sb[c][:, m * 128 : (m + 1) * 128],
                            xss[c][:],
                            start=(c == 0),
                            stop=(c == 3),
                        )
                    nc.scalar.activation(hat[:, m, :], ps[:], AF.Gelu, bias=ba_sb[m][:])


                # base[mt] = ha @ Wout[256:512] + (b_out + b[batch])  per token tile
                bases = []
                for mt in range(4):
                    ps = ps_mm.tile([128, OUTCH], fp32, tag="mm", name=f"psb{b}_{mt}")
                    nc.tensor.matmul(
                        ps[:],
                        hat[:, 0, mt * 128 : (mt + 1) * 128],
                        wout_sb[2][:],
                        start=True,
                        stop=False,
                    )
                    nc.tensor.matmul(
                        ps[:],
                        hat[:, 1, mt * 128 : (mt + 1) * 128],
                        wout_sb[3][:],
                        start=False,
                        stop=True,
                    )
                    bs = base_pool.tile(
                        [128, OUTCH], bf16, tag="bs", name=f"bs{b}_{mt}"
                    )
                    nc.vector.tensor_add(bs[:], ps[:], bvt[:])
                    bases.append(bs)

                for kq in range(KQ):
                    # pass branch for the quad: hp = gelu(Wp @ x + bp).
                    # Both hd-halves of a k share one [128,2,F] tile so the
                    # fp8 shadow copy (z-matmul moving operand) is one DVE op.
                    hps = {}
                    hp8s = {}
                    hpls = {}
                    for j in range(4):
                        k = 4 * kq + j
                        hpj = hp_pool.tile(
                            [128, 2, F], bf16, tag="hp", name=f"hp{b}{k}"
                        )
                        for m in range(2):
                            ps = ps_mm.tile(
                                [128, F], fp32, tag="mm", name=f"psp{b}{k}{m}"
                            )
                            for c in range(4):
                                nc.tensor.matmul(
                                    ps[:],
                                    wp_sb[c][:, m * 128 : (m + 1) * 128],
                                    xth[(k // 4, c)][:, (k % 4) * F : (k % 4 + 1) * F],
                                    start=(c == 0),
                                    stop=(c == 3),
                                )
                            nc.scalar.activation(
                                hpj[:, m, :], ps[:], AF.Gelu, bias=bp_sb[m][:]
                            )
                        hps[j] = hpj

                    # z for the whole quad, col-tiled: strip j holds z_{4kq+j}.
                    # The ave half is batched: one full-width matmul per chunk
                    # with the strip-placed va2 stationary covers all 4 k at
                    # once; start=True on the first also zeroes the bank (its
                    # output AP spans all 128 partitions), so no explicit
                    # zeroing op is needed. The pass half then accumulates
                    # into the strips.
                    zq = ps_z.tile([128, F], fp32, tag="zq", name=f"zq{b}{kq}")
                    for cc in range(2):
                        nc.tensor.matmul(
                            zq[:],
                            va2t[cc][:, kq * 128 : (kq + 1) * 128],
                            hat[:, cc, :],
                            start=(cc == 0),
                            stop=False,
                            skip_group_check=True,
                        )
                    cj = [(c, j) for c in range(2) for j in range(4)]
                    for nmm, (c, j) in enumerate(cj):
                        k = 4 * kq + j
                        nc.tensor.matmul(
                            zq[32 * j : 32 * j + 8, :],
                            vts[c][:, k * R : (k + 1) * R],
                            hps[j][:, c, :],
                            start=False,
                            stop=(nmm == 7),
                            tile_position=(0, 32 * j),
                            skip_group_check=True,
                        )
                    zsb = zsb_pool.tile([128, F], bf16, tag="zs", name=f"zs{b}{kq}")
                    nc.vector.tensor_scalar_mul(zsb[:], zq[:], 1.0 / 4096.0)

                    # k-pairs share a 2-bank psum tile and a [128,4,2,OUTCH]
                    # staging tile, halving DVE/ACT instruction count in the
                    # pace-critical inject+gelu chain.
                    osb2 = []
                    for jj in range(2):
                        osb2.append(
                            osb_pool.tile(
                                [128, 4, 2, OUTCH],
                                bf16,
                                tag="osb",
                                name=f"o{b}{kq}{jj}",
                            )
                        )
                    for mt in range(4):
                        po2s = []
                        for jj in range(2):
                            po2 = ps_o.tile(
                                [128, 2, OUTCH], fp32, tag="po", name=f"po{b}{kq}{mt}{jj}"
                            )
                            for sub in range(2):
                                j = 2 * jj + sub
                                for cc in range(2):
                                    nc.tensor.matmul(
                                        po2[:, sub, :],
                                        hps[j][:, cc, mt * 128 : (mt + 1) * 128],
                                        wout_sb[cc][:],
                                        start=(cc == 0),
                                        stop=False,
                                    )
                            po2s.append(po2)
                        for j in range(4):
                            nc.tensor.matmul(
                                po2s[j // 2][:, j % 2, :],
                                zsb[32 * j : 32 * j + 8, mt * 128 : (mt + 1) * 128],
                                u2t[
                                    32 * j : 32 * j + 8,
                                    kq * OUTCH : (kq + 1) * OUTCH,
                                ],
                                start=False,
                                stop=True,
                                tile_position=(32 * j, 0),
                            )
                        bcast = (
                            bases[mt][:]
                            .unsqueeze(1)
                            .broadcast_to((128, 2, OUTCH))
                        )
                        for jj in range(2):
                            ta2 = ta_pool.tile(
                                [128, 2, OUTCH],
                                fp32,
                                tag="ta",
                                name=f"ta{b}{kq}{mt}{jj}",
                            )
                            nc.vector.tensor_add(ta2[:], po2s[jj][:], bcast)
                            nc.scalar.activation(
                                osb2[jj][:, mt, :, :],
                                ta2[:],
                                AF.Gelu,
                                scale=1.0 / 16.0,
                            )
                    for j in range(4):
                        k = 4 * kq + j
                        oview = out_d[b, k].rearrange("(m p) o -> p m o", p=128)
                        for hh in range(2):
                            nc.sync.dma_start(
                                out=oview[:, 2 * hh : 2 * hh + 2, :],
                                in_=osb2[j // 2][:, 2 * hh : 2 * hh + 2, j % 2, :],
                            )

            # `unroll` bodies per hardware-loop iteration: amortizes the
            # For_i all-engine barrier (head/tail pipeline drain) over
            # `unroll` iterations.
            assert loop_n % unroll == 0, (loop_n, unroll)
            n_hw = loop_n // unroll
            loop_cm = tc.For_i(0, n_hw, 1) if n_hw > 1 else contextlib.nullcontext()
            with loop_cm:
                for _u in range(unroll):
                    if _u == 0:
                        # only the first body per hw-iteration needs the
                        # HAM-warm: later bodies chain PE work back-to-back
                        emit_prewarm()
                    for b in range(BPC):
                        emit_batch(b)
    nc.compile()
    return nc


def _prep_inputs(x, u, v, b, W_pass, b_pass, W_ave, b_ave, W_out, b_out):
    x = np.asarray(x, dtype=np.float32)
    u = np.asarray(u, dtype=np.float32)
    v = np.asarray(v, dtype=np.float32)
    b = np.asarray(b, dtype=np.float32)

    # split-fp8 x: x = x_hi + x_lo (exact in fp32 up to x_lo's own e4m3
    # rounding). Layout [pair, p, cslot, kf]: inch = pair*256 + cslot*128 + p.
    xb = np.ascontiguousarray(
        x.reshape(B, K * F, INCH).astype(BF16).transpose(0, 2, 1)
    )
    xT = x.reshape(B, K * F, INCH).transpose(0, 2, 1)  # [B, INCH, KF]
    xr = xT.reshape(B, 2, 2, 128, K * F).transpose(0, 1, 3, 2, 4)
    xhi = xr.astype(F8)
    xlo = (xr - xhi.astype(np.float32)).astype(F8)
    xsum = np.ascontiguousarray(x.sum(axis=1).transpose(0, 2, 1)).astype(BF16)
    # v unscaled fp8; the 1/(INCH*R) lands on the z psum->sbuf copy
    vs = v.transpose(0, 2, 1, 3)  # [B, INCH, K, R]
    vb = np.ascontiguousarray(vs[:, :HD].reshape(B, HD, K * R)).astype(BF16)
    # va2[b, i, kq*128 + 32*j + r] = v[b, HD + i, 4*kq + j, r]
    va2 = np.zeros((B, HD, KQ * 128), dtype=np.float32)
    for kq in range(KQ):
        for j in range(4):
            va2[:, :, kq * 128 + 32 * j : kq * 128 + 32 * j + 8] = vs[
                :, HD:, 4 * kq + j, :
            ]
    va2 = va2.astype(BF16)
    # u2[b, 32*j + r, kq*OUTCH + o] = 16 * u[b, 4*kq + j, o, r]  (the out
    # psum runs at 16x; the final gelu rescales by 1/16)
    u2 = np.zeros((B, 128, KQ * OUTCH), dtype=BF16)
    ut = (u * 16.0).transpose(0, 1, 3, 2).astype(BF16)  # [B, K, R, OUTCH]
    for kq in range(KQ):
        for j in range(4):
            u2[:, 32 * j : 32 * j + 8, kq * OUTCH : (kq + 1) * OUTCH] = ut[
                :, 4 * kq + j
            ]
    bvec = (
        16.0 * (np.asarray(b_out, np.float32)[None, :] + b[:, 0, 0, :])
    ).astype(BF16)
    bvec128 = np.ascontiguousarray(
        np.broadcast_to(bvec[:, None, :], (B, 128, OUTCH))
    )
    # W_pass split-fp8 (x16): [pair, p, cslot, hd]
    wpT = 16.0 * np.asarray(W_pass, dtype=np.float32).T  # [INCH, HD]
    wpr = wpT.reshape(2, 2, 128, HD).transpose(0, 2, 1, 3)
    wpassh = wpr.astype(F8)
    wpassl = (wpr - wpassh.astype(np.float32)).astype(F8L)
    waveT = np.ascontiguousarray(np.asarray(W_ave, dtype=np.float32).T / K).astype(BF16)
    # W_out (x16): pass half split-fp8 [p, cc, outch]; ave half bf16
    woT = 16.0 * np.asarray(W_out, dtype=np.float32).T  # [INCH, OUTCH]
    wor = woT[:HD].reshape(2, 128, OUTCH).transpose(1, 0, 2)
    wouth = wor.astype(F8)
    woutl = (wor - wouth.astype(np.float32)).astype(F8L)
    woutT = np.ascontiguousarray(woT).astype(BF16)
    bp = np.asarray(b_pass, dtype=np.float32).reshape(HD, 1)
    ba = np.asarray(b_ave, dtype=np.float32).reshape(HD, 1)

    in_maps = []
    for i in range(N_CORES):
        sl = slice(i * BPC, (i + 1) * BPC)
        in_maps.append(
            dict(
                x=np.ascontiguousarray(xb[sl]),
                xhi=np.ascontiguousarray(xhi[sl]),
                xlo=np.ascontiguousarray(xlo[sl]),
                xsum=np.ascontiguousarray(xsum[sl]),
                v=np.ascontiguousarray(vb[sl]),
                va2=np.ascontiguousarray(va2[sl]),
                u2=np.ascontiguousarray(u2[sl]),
                bvec=np.ascontiguousarray(bvec128[sl]),
                wpassT=np.ascontiguousarray(
                    np.asarray(W_pass, dtype=np.float32).T
                ).astype(BF16),
                wpassh=wpassh,
                wpassl=wpassl,
                waveT=waveT,
                wouth=wouth,
                woutl=woutl,
                woutT=woutT,
                bpass=bp,
                bave=ba,
            )
        )
    return in_maps


def run(inputs, trace=False, loop_n=1, **spmd_kwargs):
    from concourse.bass_utils import run_bass_kernel_spmd

    key = ("nc", loop_n)
    if key not in _CACHE:
        _CACHE[key] = _build_bass(loop_n)
    nc = _CACHE[key]
    in_maps = _prep_inputs(**inputs)
    res = run_bass_kernel_spmd(
        nc, in_maps, list(range(N_CORES)), trace=trace, **spmd_kwargs
    )
    out = np.concatenate(
        [np.asarray(res.results[i]["out"]).astype(np.float32) for i in range(N_CORES)],
        axis=0,
    ).reshape(B, K, F, OUTCH)
    return out, res


def kernel(**inputs):
    out, _ = run(inputs, trace=False)
    return out

